# revision 1
# baseline (speedup 1.0000x reference)
"""Trainium2 Bass kernel for nn_CCA_Block (cross-channel attention block).

Reference computation (per batch element, B=8 sharded one-per-core):
    q = relu(x1 @ Wq); k = relu(x1 @ Wk); v = relu(x2 @ Wv)      # 1x1 convs
    scores[c,h,g] = scale * sum_w q[h,w,c] * k[g,w,c]
    attn = softmax(scores, axis=g)
    o[h,w,c] = sum_g attn[c,h,g] * v[g,w,c]
    g = sigmoid(o @ Ws + bs)
    g = gamma * (g - mu) / sqrt(var + eps) + beta
    out = x1 + x2 * g

Sharding: data-parallel over batch across the 8 NeuronCores (batch b -> core b).

Per-core dataflow (matmuls in bf16 with fp32 PSUM accumulate; transposes are
PE transpose-mode matmuls batched 4-wide into bf16 PSUM). The V-conv and
QK-conv pipelines are emitted interleaved to keep the PE array dense:
  V:  x2 w-major cast-DMA -> PE transpose -> x2T -> PE conv -> relu -> v_sb
  QK: x1 h-major cast-DMA -> PE transpose -> x1T -> fused q|k conv (N=256
      moving operand [Wq|Wk]) -> one relu evac -> qk_sb [w, h*256+qk*128+c]
  A:  per channel c: scoresT = kT_c' qT_c (PE) -> exp (ACT, scale folded)
      -> o_unnorm = E' V_c and Z = E' 1 (PE) -> 1/Z (DVE)
      -> o = o_unnorm * (1/Z bcast) + delta  (Ws^T delta = bs)
  G:  o_sb -> PE transpose -> conv with Ws -> sigmoid (ACT)
      -> BN affine (host-folded a,b) -> t = x2*g -> t += x1 (DMA accumulate)
"""

import numpy as np
import ml_dtypes

B, H, W, C = 8, 128, 128, 128
N_CORES = 8
BN_EPS = 1e-3

_BUILD_CACHE: dict = {}


def _build_program(scale_val: float, delta: tuple, bias_via_dve: bool, b_zero: bool):
    """Emit + compile the per-core Bass program. All cores run the identical
    program on their own batch slice."""
    import concourse.bacc as bacc
    import concourse.mybir as mybir
    import concourse.tile as tile

    fp32 = mybir.dt.float32
    bf16 = mybir.dt.bfloat16
    AF = mybir.ActivationFunctionType
    OP = mybir.AluOpType
    delta_zero = all(d == 0.0 for d in delta)

    nc = bacc.Bacc("TRN2", target_bir_lowering=False, debug=False,
                   enable_asserts=False)

    x1_d = nc.dram_tensor("x1", [H, W, C], fp32, kind="ExternalInput")
    x2_d = nc.dram_tensor("x2", [H, W, C], fp32, kind="ExternalInput")
    wqk_d = nc.dram_tensor("wqk", [C, 2 * C], bf16, kind="ExternalInput")
    wv_d = nc.dram_tensor("wv", [C, C], bf16, kind="ExternalInput")
    ws_d = nc.dram_tensor("ws", [C, C], bf16, kind="ExternalInput")
    ones_d = nc.dram_tensor("ones_col", [C, 1], bf16, kind="ExternalInput")
    ident_d = nc.dram_tensor("ident", [C, C], bf16, kind="ExternalInput")
    arep_d = nc.dram_tensor("a_rep", [C, 4 * C], bf16, kind="ExternalInput")
    brep_d = nc.dram_tensor("b_rep", [C, 4 * C], bf16, kind="ExternalInput")
    bsrep_d = nc.dram_tensor("bs_rep", [C, 4 * C], fp32, kind="ExternalInput")
    out_d = nc.dram_tensor("out", [H, W, C], fp32, kind="ExternalOutput")

    x1_ap, x2_ap, out_ap = x1_d.ap(), x2_d.ap(), out_d.ap()

    with tile.TileContext(nc) as tc:
        with (
            # persistent single-buffer pools
            tc.tile_pool(name="wts", bufs=1) as p_wts,
            tc.tile_pool(name="qkv", bufs=1) as p_qkv,
            tc.tile_pool(name="obuf", bufs=1) as p_o,
            # streaming pools
            tc.tile_pool(name="xcast", bufs=3) as p_xcast,
            tc.tile_pool(name="xT", bufs=6) as p_xT,
            tc.tile_pool(name="eexp", bufs=6) as p_e,
            tc.tile_pool(name="rz", bufs=6) as p_rz,
            tc.tile_pool(name="gres", bufs=4) as p_g,
            tc.tile_pool(name="x2f", bufs=4) as p_x2f,
            tc.tile_pool(name="outt", bufs=4) as p_out,
            # psum: shared full-bank fp32 tag (6) + bf16 transpose tag (2)
            tc.tile_pool(name="psA", bufs=6, space="PSUM") as ps_a,
            tc.tile_pool(name="psT", bufs=2, space="PSUM") as ps_t,
        ):
            # ---- constants ----
            wqk = p_wts.tile([C, 2 * C], bf16, tag="wqk")
            wv = p_wts.tile([C, C], bf16, tag="wv")
            ws = p_wts.tile([C, C], bf16, tag="ws")
            ones = p_wts.tile([C, 1], bf16, tag="ones")
            ident = p_wts.tile([C, C], bf16, tag="ident")
            arep = p_wts.tile([C, 4 * C], bf16, tag="arep")
            nc.sync.dma_start(wqk[:], wqk_d.ap())
            nc.sync.dma_start(wv[:], wv_d.ap())
            nc.sync.dma_start(ws[:], ws_d.ap())
            nc.sync.dma_start(ones[:], ones_d.ap())
            nc.sync.dma_start(ident[:], ident_d.ap())
            nc.sync.dma_start(arep[:], arep_d.ap())
            if not b_zero:
                brep = p_wts.tile([C, 4 * C], bf16, tag="brep")
                nc.sync.dma_start(brep[:], brep_d.ap())
            if bias_via_dve:
                bsrep = p_wts.tile([C, 4 * C], fp32, tag="bsrep")
                nc.sync.dma_start(bsrep[:], bsrep_d.ap())

            # persistent big buffers (bf16): free-axis layouts noted
            qk_sb = p_qkv.tile([W, H * 2 * C], bf16, tag="qk")  # [w,h*256+s*128+c]
            # v plus a trailing ones-column block: column W*C+c == 1.0 so a
            # single N=129 matmul computes both o_unnorm and the softmax
            # denominator Z (as output column 128)
            v_sb = p_qkv.tile([H, W * C + C], bf16, tag="v")    # [g, w*128+c]
            nc.vector.memset(v_sb[:, W * C :], 1.0)
            o_sb = p_o.tile([H, C * W], bf16, tag="o")          # [h, c*128+w]

            def transpose4(src_fn, evac_engine):
                """4 PE tile-transposes into one bf16 PSUM bank + wide evac.
                src_fn(j) -> [128,128] bf16 SBUF AP. Returns SBUF tile
                [128, 512] holding the 4 transposed tiles."""
                pst = ps_t.tile([C, 512], bf16, tag="pst")
                for j in range(4):
                    nc.tensor.matmul(
                        pst[:, j * C : (j + 1) * C], src_fn(j), ident[:],
                        is_transpose=True, start=(j == 0), stop=(j == 3),
                    )
                xt = p_xT.tile([C, 512], bf16, tag="xT")
                if evac_engine == "act":
                    nc.scalar.activation(xt[:], pst[:], AF.Copy)
                elif evac_engine == "dve":
                    nc.vector.tensor_copy(xt[:], pst[:])
                else:  # split halves across both engines in parallel
                    nc.scalar.activation(xt[:, :256], pst[:, :256], AF.Copy)
                    nc.vector.tensor_copy(xt[:, 256:], pst[:, 256:])
                return xt

            # ===== Phases V and QK, interleaved per 4-pixel group =====
            for p0 in range(0, W, 4):
                # --- V group: x2 -> x2T -> v ---
                xc2 = p_xcast.tile([H, 4 * C], bf16, tag="xc2")
                nc.gpsimd.dma_start(xc2[:], x2_ap[:, p0 : p0 + 4, :])
                x2T = transpose4(
                    lambda j: xc2[:, j * C : (j + 1) * C], "dve"
                )
                psv = ps_a.tile([H, 512], fp32, tag="ps")
                for j in range(4):
                    nc.tensor.matmul(
                        psv[:, j * C : (j + 1) * C],
                        x2T[:, j * C : (j + 1) * C], wv[:],
                        start=(j == 0), stop=(j == 3),
                    )
                nc.scalar.activation(
                    v_sb[:, p0 * C : (p0 + 4) * C], psv[:], AF.Relu
                )

                # --- QK group: x1 -> x1T -> fused q|k conv ---
                xc = p_xcast.tile([W, 4 * C], bf16, tag="xc")
                src = x1_ap[p0 : p0 + 4].rearrange("hh w c -> w hh c")
                nc.gpsimd.dma_start(xc[:], src)
                x1T = transpose4(
                    lambda j: xc[:, j * C : (j + 1) * C], "act"
                )
                for s in range(2):  # two 2-h conv subgroups
                    psqk = ps_a.tile([W, 512], fp32, tag="ps")
                    for t in range(2):
                        j = 2 * s + t
                        nc.tensor.matmul(
                            psqk[:, t * 256 : (t + 1) * 256],
                            x1T[:, j * C : (j + 1) * C], wqk[:],
                            start=(t == 0), stop=(t == 1),
                        )
                    h2 = p0 + 2 * s
                    if s == 0:
                        nc.vector.tensor_scalar(
                            qk_sb[:, h2 * 2 * C : (h2 + 2) * 2 * C],
                            psqk[:], 0.0, None, OP.max,
                        )
                    else:
                        nc.scalar.activation(
                            qk_sb[:, h2 * 2 * C : (h2 + 2) * 2 * C],
                            psqk[:], AF.Relu,
                        )

            # ============ Phase A: attention over channels ============
            # qk_sb free layout: h*256 + s*128 + c  (s=0 -> q, s=1 -> k)
            qk4 = qk_sb[:].rearrange("w (h s c) -> w h s c", s=2, c=C)
            groups = [(c0, min(3, C - c0)) for c0 in range(0, C, 3)]
            for c0, gs in groups:
                pss = ps_a.tile([H, gs * H], fp32, tag="ps")
                for j in range(gs):
                    c = c0 + j
                    nc.tensor.matmul(
                        pss[:, j * H : (j + 1) * H],
                        qk4[:, :, 1, c], qk4[:, :, 0, c],
                        start=(j == 0), stop=(j == gs - 1),
                    )
                e4 = p_e.tile([H, gs * H], bf16, tag="e4")
                for j in range(gs):
                    nc.scalar.activation(
                        e4[:, j * H : (j + 1) * H],
                        pss[:, j * H : (j + 1) * H], AF.Exp, scale=scale_val,
                    )
                pso = ps_a.tile([H, gs * 129], fp32, tag="ps")
                for j in range(gs):
                    c = c0 + j
                    nc.tensor.matmul(
                        pso[:, j * 129 : (j + 1) * 129],
                        e4[:, j * H : (j + 1) * H],
                        v_sb[:, c : c + W * C + 1 : C],
                        start=(j == 0), stop=(j == gs - 1),
                    )
                po = pso[:].rearrange("h (j x) -> h j x", x=129)
                rz = p_rz.tile([H, gs], fp32, tag="rz")
                nc.vector.reciprocal(rz[:], po[:, :, 128])
                if delta_zero:
                    # wide normalize: o = o_unnorm * (1/Z) with 1/Z
                    # broadcast along w via a stride-0 AP
                    rzb = rz[:].unsqueeze(2).broadcast_to([H, gs, C])
                    nc.vector.tensor_tensor(
                        o_sb[:, c0 * W : (c0 + gs) * W],
                        po[:, :, 0:128], rzb, OP.mult,
                    )
                else:
                    for j in range(gs):
                        c = c0 + j
                        dst = o_sb[:, c * W : (c + 1) * W]
                        src_ap = pso[:, j * 129 : j * 129 + 128]
                        if (c0 // 3) % 2 == 0:
                            nc.scalar.activation(
                                dst, src_ap, AF.Copy,
                                bias=float(delta[c]), scale=rz[:, j : j + 1],
                            )
                        else:
                            nc.vector.tensor_scalar(
                                dst, src_ap, rz[:, j : j + 1], float(delta[c]),
                                OP.mult, OP.add,
                            )

            # ============ Phase G: o -> oT -> conv -> sigmoid/BN/residual ====
            o3 = o_sb[:].rearrange("h (c w) -> h c w", w=W)
            for w0 in range(0, W, 4):
                oT = transpose4(lambda j: o3[:, :, w0 + j], "split")
                psg = ps_a.tile([H, 512], fp32, tag="ps")
                for j in range(4):
                    nc.tensor.matmul(
                        psg[:, j * C : (j + 1) * C],
                        oT[:, j * H : (j + 1) * H], ws[:],
                        start=(j == 0), stop=(j == 3),
                    )
                if bias_via_dve:
                    nc.vector.tensor_tensor(psg[:], psg[:], bsrep[:], OP.add)
                g4 = p_g.tile([H, 512], bf16, tag="g4")
                nc.scalar.activation(g4[:], psg[:], AF.Sigmoid)
                gbn = p_g.tile([H, 512], bf16, tag="gbn")
                nc.vector.tensor_tensor(gbn[:], g4[:], arep[:], OP.mult)
                if not b_zero:
                    nc.vector.tensor_tensor(gbn[:], gbn[:], brep[:], OP.add)
                x2f = p_x2f.tile([H, 512], fp32, tag="x2f")
                nc.sync.dma_start(x2f[:], x2_ap[:, w0 : w0 + 4, :])
                t4 = p_out.tile([H, 512], fp32, tag="t4")
                if w0 % 8 == 0:
                    nc.vector.tensor_tensor(t4[:], x2f[:], gbn[:], OP.mult)
                else:
                    nc.gpsimd.tensor_tensor(t4[:], x2f[:], gbn[:], OP.mult)
                # residual add: t4 += x1 via SWDGE accumulate DMA
                nc.gpsimd.dma_start(
                    t4[:], x1_ap[:, w0 : w0 + 4, :], accum_op=OP.add
                )
                nc.sync.dma_start(out_ap[:, w0 : w0 + 4, :], t4[:])

    nc.compile()
    return nc


def _prepare(inputs):
    """Host-side prep: derived small tensors + baked scalars."""
    x1 = np.ascontiguousarray(np.asarray(inputs["x1"], dtype=np.float32))
    x2 = np.ascontiguousarray(np.asarray(inputs["x2"], dtype=np.float32))
    Wq = np.asarray(inputs["Wq"], dtype=np.float32)
    Wk = np.asarray(inputs["Wk"], dtype=np.float32)
    Wv = np.asarray(inputs["Wv"], dtype=np.float32)
    Ws = np.asarray(inputs["Ws"], dtype=np.float32)
    bs = np.asarray(inputs["bs"], dtype=np.float32)
    scale = float(np.asarray(inputs["scale"]).reshape(-1)[0])
    gamma = np.asarray(inputs["gamma"], dtype=np.float32)
    beta = np.asarray(inputs["beta"], dtype=np.float32)
    mu = np.asarray(inputs["mu"], dtype=np.float32)
    var = np.asarray(inputs["var"], dtype=np.float32)

    a = gamma / np.sqrt(var + BN_EPS)
    b = beta - mu * a
    b_zero = bool(np.all(b == 0.0))

    # fold the sigmoid bias bs into o:  o' = o + delta with Ws^T delta = bs
    bias_via_dve = False
    delta = np.zeros(C, dtype=np.float64)
    if np.any(bs != 0.0):
        try:
            delta = np.linalg.solve(Ws.astype(np.float64).T, bs.astype(np.float64))
            resid = np.abs(Ws.T @ delta.astype(np.float32) - bs).max()
            if not np.isfinite(delta).all() or resid > 1e-5 * (1 + np.abs(bs).max()):
                raise np.linalg.LinAlgError("bad solve")
        except np.linalg.LinAlgError:
            delta = np.zeros(C, dtype=np.float64)
            bias_via_dve = True

    bf = ml_dtypes.bfloat16
    consts = {
        "wqk": np.concatenate([Wq, Wk], axis=1).astype(bf),
        "wv": Wv.astype(bf),
        "ws": Ws.astype(bf),
        "ones_col": np.ones((C, 1), dtype=bf),
        "ident": np.eye(C, dtype=bf),
        "a_rep": np.tile(a, (C, 4)).astype(bf),
        "b_rep": np.tile(b, (C, 4)).astype(bf),
        "bs_rep": np.tile(bs, (C, 4)).astype(np.float32),
    }
    key = (scale, tuple(np.round(delta, 12)), bias_via_dve, b_zero)
    return x1, x2, consts, key, scale, delta, bias_via_dve, b_zero


def _get_nc(key, scale, delta, bias_via_dve, b_zero):
    if key not in _BUILD_CACHE:
        _BUILD_CACHE[key] = _build_program(scale, delta, bias_via_dve, b_zero)
    return _BUILD_CACHE[key]


def run(inputs, trace: bool = False):
    from concourse.bass_utils import run_bass_kernel_spmd

    x1, x2, consts, key, scale, delta, bias_via_dve, b_zero = _prepare(inputs)
    nc = _get_nc(key, scale, delta, bias_via_dve, b_zero)

    in_maps = []
    for core in range(N_CORES):
        m = dict(consts)
        m["x1"] = x1[core]
        m["x2"] = x2[core]
        in_maps.append(m)

    res = run_bass_kernel_spmd(
        nc, in_maps, core_ids=list(range(N_CORES)), trace=trace
    )
    out = np.stack([res.results[i]["out"] for i in range(N_CORES)], axis=0)
    return out.astype(np.float32), res


def kernel(**inputs) -> np.ndarray:
    out, _ = run(inputs, trace=False)
    return out



# revision 10
# speedup vs baseline: 1.0015x; 1.0015x over previous
"""Trainium2 Bass kernel for nn_CCA_Block (cross-channel attention block).

Reference computation (per batch element, B=8 sharded one-per-core):
    q = relu(x1 @ Wq); k = relu(x1 @ Wk); v = relu(x2 @ Wv)      # 1x1 convs
    scores[c,h,g] = scale * sum_w q[h,w,c] * k[g,w,c]
    attn = softmax(scores, axis=g)
    o[h,w,c] = sum_g attn[c,h,g] * v[g,w,c]
    g = sigmoid(o @ Ws + bs)
    g = gamma * (g - mu) / sqrt(var + eps) + beta
    out = x1 + x2 * g

Sharding: data-parallel over batch across the 8 NeuronCores (batch b -> core b).

Per-core dataflow (v2 — engine-balanced):
  All HBM traffic via HWDGE fp32 loads + wide engine casts (no SWDGE: the
  Q7 descriptor-gen cost of cast-DMAs dominated the old kernel's GpSimd).
  QK: x1 staged w-major -> cast bf16 -> PE transpose -> fused q|k conv
      (wqk moving, N=256) -> relu evac scattered to qk_sb [w, (s,c,h)]
      so per-channel q_c/k_c slices are contiguous (FWL-friendly lhsT).
  V:  x2 staged h-major -> cast into persistent x2b chunks -> PE transpose
      -> conv (wv moving) -> relu evac scattered to v_sb [g, c*(W+1)+w]
      with a per-channel ones column at w=W (softmax denominator trick).
  A:  4-channel score groups per PSUM bank -> ONE wide exp per bank
      (batches the 352-cycle ACT fixed cost) -> e tiles [g, (c,h)]
      -> per-channel o matmul with contiguous N=129 rhs -> 1/Z (DVE)
      -> wide normalize into o_sb [h, (w,c)].
  G:  oT transpose (contiguous src) -> conv with Ws -> sigmoid -> BN
      affine -> t = x2b*g + x1 (x1 re-read as fp32 HWDGE prefetch)
      -> out store.
"""

import numpy as np
import ml_dtypes

B, H, W, C = 8, 128, 128, 128
N_CORES = 8
BN_EPS = 1e-3
W1 = W + 1  # v row width incl ones column

_BUILD_CACHE: dict = {}


def _build_program(scale_val: float, delta: tuple, bias_via_dve: bool, b_zero: bool):
    """Emit + compile the per-core Bass program. All cores run the identical
    program on their own batch slice."""
    import concourse.bacc as bacc
    import concourse.mybir as mybir
    import concourse.tile as tile

    fp32 = mybir.dt.float32
    bf16 = mybir.dt.bfloat16
    AF = mybir.ActivationFunctionType
    OP = mybir.AluOpType
    delta_zero = all(d == 0.0 for d in delta)

    nc = bacc.Bacc("TRN2", target_bir_lowering=False, debug=False,
                   enable_asserts=False)

    x1_d = nc.dram_tensor("x1", [H, W, C], fp32, kind="ExternalInput")
    x2_d = nc.dram_tensor("x2", [H, W, C], fp32, kind="ExternalInput")
    wqk_d = nc.dram_tensor("wqk", [C, 2 * C], bf16, kind="ExternalInput")
    wv_d = nc.dram_tensor("wv", [C, C], bf16, kind="ExternalInput")
    ws_d = nc.dram_tensor("ws", [C, C], bf16, kind="ExternalInput")
    ident_d = nc.dram_tensor("ident", [C, C], bf16, kind="ExternalInput")
    arep_d = nc.dram_tensor("a_rep", [C, 4 * C], bf16, kind="ExternalInput")
    brep_d = nc.dram_tensor("b_rep", [C, 4 * C], bf16, kind="ExternalInput")
    bsrep_d = nc.dram_tensor("bs_rep", [C, 4 * C], fp32, kind="ExternalInput")
    out_d = nc.dram_tensor("out", [H, W, C], fp32, kind="ExternalOutput")

    x1_ap, x2_ap, out_ap = x1_d.ap(), x2_d.ap(), out_d.ap()

    with tile.TileContext(nc) as tc:
        with (
            # persistent single-buffer pools
            tc.tile_pool(name="wts", bufs=1) as p_wts,
            tc.tile_pool(name="qkv", bufs=1) as p_qkv,
            tc.tile_pool(name="obuf", bufs=1) as p_o,
            tc.tile_pool(name="x2res", bufs=16) as p_x2b,
            # streaming pools
            tc.tile_pool(name="stage", bufs=3) as p_stage,     # fp32 staging
            tc.tile_pool(name="xcast", bufs=3) as p_xcast,     # bf16 cast chunks
            tc.tile_pool(name="xT", bufs=4) as p_xT,
            tc.tile_pool(name="eexp", bufs=4) as p_e,
            tc.tile_pool(name="rz", bufs=6) as p_rz,
            tc.tile_pool(name="gres", bufs=4) as p_g,
            tc.tile_pool(name="x1f", bufs=2) as p_x1f,         # fp32 residual
            tc.tile_pool(name="outt", bufs=2) as p_out,
            # psum
            tc.tile_pool(name="psA", bufs=5, space="PSUM") as ps_a,
            tc.tile_pool(name="psT", bufs=3, space="PSUM") as ps_t,
        ):
            # ---- constants ----
            wqk = p_wts.tile([C, 2 * C], bf16, tag="wqk")
            wv = p_wts.tile([C, C], bf16, tag="wv")
            ws = p_wts.tile([C, C], bf16, tag="ws")
            ident = p_wts.tile([C, C], bf16, tag="ident")
            arep = p_wts.tile([C, 4 * C], bf16, tag="arep")
            nc.sync.dma_start(wqk[:], wqk_d.ap())
            nc.sync.dma_start(wv[:], wv_d.ap())
            nc.sync.dma_start(ws[:], ws_d.ap())
            nc.sync.dma_start(ident[:], ident_d.ap())
            nc.sync.dma_start(arep[:], arep_d.ap())
            if not b_zero:
                brep = p_wts.tile([C, 4 * C], bf16, tag="brep")
                nc.sync.dma_start(brep[:], brep_d.ap())
            if bias_via_dve:
                bsrep = p_wts.tile([C, 4 * C], fp32, tag="bsrep")
                nc.sync.dma_start(bsrep[:], bsrep_d.ap())

            # persistent big buffers
            # qk_sb free layout: s*C*H + c*H + h  (s=0 -> q, s=1 -> k)
            qk_sb = p_qkv.tile([W, 2 * C * H], bf16, tag="qk")
            # v_sb free layout: c*(W+1) + w; column w=W is 1.0 per channel
            v_sb = p_qkv.tile([H, C * W1], bf16, tag="v")
            nc.vector.memset(v_sb[:].rearrange("h (c w1) -> h c w1", w1=W1)[:, :, W], 1.0)
            # o_sb free layout: w*C + c (pixel-major)
            o_sb = p_o.tile([H, W * C], bf16, tag="o")
            # x2 bf16 resident chunks: chunk i holds x2[:, 8i:8i+8, :]
            x2b = [
                p_x2b.tile([H, 8 * C], bf16, tag="x2b", name=f"x2b{i}")
                for i in range(16)
            ]

            qk4 = qk_sb[:].rearrange("w (s c h) -> w s c h", s=2, c=C)
            v3 = v_sb[:].rearrange("h (c w1) -> h c w1", w1=W1)
            o3 = o_sb[:].rearrange("h (w c) -> h w c", c=C)

            def transpose4(src_fn, pst_dtype=bf16):
                """4 PE tile-transposes into one bf16 PSUM bank.
                src_fn(j) -> [128,128] bf16 SBUF AP. Returns PSUM tile."""
                pst = ps_t.tile([C, 512], pst_dtype, tag="pst")
                for j in range(4):
                    nc.tensor.matmul(
                        pst[:, j * C : (j + 1) * C], src_fn(j), ident[:],
                        is_transpose=True, start=(j == 0), stop=(j == 3),
                    )
                return pst

            def evac(dst, src, engine):
                if engine == "act":
                    nc.scalar.activation(dst, src, AF.Copy)
                elif engine == "dve":
                    nc.vector.tensor_copy(dst, src)
                else:
                    nc.gpsimd.tensor_copy(dst, src)

            def relu_evac(dst, src, engine):
                if engine == "act":
                    nc.scalar.activation(dst, src, AF.Relu)
                elif engine == "dve":
                    nc.vector.tensor_scalar(dst, src, 0.0, None, OP.max)
                else:
                    nc.gpsimd.tensor_scalar(dst, src, 0.0, None, OP.max)

            # ===== Phase QK: x1 -> x1T -> fused q|k conv =====
            # also interleaves the x2 load+cast (into x2b) for phase V.
            for i in range(16):
                h0 = 8 * i
                stg = p_stage.tile([W, 8 * C], fp32, tag="stg")
                nc.sync.dma_start(
                    stg[:], x1_ap[h0 : h0 + 8].rearrange("hh w c -> w hh c")
                )
                xb = p_xcast.tile([W, 8 * C], bf16, tag="xb")
                evac(xb[:], stg[:], "gps")

                # x2 prefetch for phase V (natural h-major layout)
                stg2 = p_stage.tile([H, 8 * C], fp32, tag="stg")
                nc.sync.dma_start(stg2[:], x2_ap[:, h0 : h0 + 8, :])
                evac(x2b[i][:], stg2[:], "gps")

                for j2 in range(2):  # two 4-h subgroups
                    hh = 4 * j2
                    pst = transpose4(lambda j: xb[:, (hh + j) * C : (hh + j + 1) * C])
                    xt = p_xT.tile([C, 512], bf16, tag="xT")
                    evac(xt[:], pst[:], "act")
                    for s2 in range(2):  # two 2-h conv banks
                        psqk = ps_a.tile([W, 512], fp32, tag="ps")
                        for t in range(2):
                            nc.tensor.matmul(
                                psqk[:, t * 256 : (t + 1) * 256],
                                xt[:, (2 * s2 + t) * C : (2 * s2 + t + 1) * C],
                                wqk[:], start=(t == 0), stop=(t == 1),
                            )
                        h = h0 + hh + 2 * s2
                        # scatter: [w, t(h), s, c] -> qk_sb[w, s*CH + c*H + (h+t)]
                        dst = qk4[:, :, :, h : h + 2].rearrange("w s c t -> w t s c")
                        src = psqk[:].rearrange("w (t s c) -> w t s c", t=2, s=2)
                        relu_evac(dst, src, ("dve", "act", "dve", "act")[2 * j2 + s2])

            # ===== Phase V: x2b -> x2T -> v conv =====
            for i in range(16):
                w0 = 8 * i
                for j2 in range(2):
                    ww = 4 * j2
                    pst = transpose4(
                        lambda j: x2b[i][:, (ww + j) * C : (ww + j + 1) * C]
                    )
                    xt = p_xT.tile([C, 512], bf16, tag="xT")
                    evac(xt[:], pst[:], "act" if j2 == 0 else "dve")
                    psv = ps_a.tile([H, 512], fp32, tag="ps")
                    for j in range(4):
                        nc.tensor.matmul(
                            psv[:, j * C : (j + 1) * C],
                            xt[:, j * C : (j + 1) * C], wv[:],
                            start=(j == 0), stop=(j == 3),
                        )
                    # scatter: [h, j(w), c] -> v_sb[h, c*W1 + (w0+ww+j)]
                    dst = v3[:, :, w0 + ww : w0 + ww + 4].rearrange("h c j -> h j c")
                    src = psv[:].rearrange("h (j c) -> h j c", j=4)
                    relu_evac(dst, src, "dve" if j2 == 0 else "act")

            # ===== Phase A: attention over channels =====
            e_tiles = {}  # sg -> tile
            o_groups = [(c0, min(3, C - c0)) for c0 in range(0, C, 3)]
            next_og = 0

            def emit_o_group(c0, gs):
                pso = ps_a.tile([H, gs * 129], fp32, tag="ps")
                for j in range(gs):
                    c = c0 + j
                    et = e_tiles[c // 4]
                    nc.tensor.matmul(
                        pso[:, j * 129 : (j + 1) * 129],
                        et[:, (c % 4) * H : (c % 4 + 1) * H],
                        v_sb[:, c * W1 : (c + 1) * W1],
                        start=(j == 0), stop=(j == gs - 1),
                    )
                po = pso[:].rearrange("h (j x) -> h j x", x=129)
                rz = p_rz.tile([H, gs], fp32, tag="rz")
                nc.vector.reciprocal(rz[:], po[:, :, 128])
                if delta_zero:
                    # wide normalize: o = o_unnorm * (1/Z), 1/Z broadcast
                    # along w; dst is o_sb pixel-major [h, (w, c0+j)]
                    dst = o3[:, :, c0 : c0 + gs]
                    src = pso[:].rearrange("h (j x) -> h x j", x=129)[:, 0:W, :]
                    rzb = rz[:].unsqueeze(1).broadcast_to([H, W, gs])
                    nc.vector.tensor_tensor(dst, src, rzb, OP.mult)
                else:
                    for j in range(gs):
                        c = c0 + j
                        dst = o3[:, :, c]
                        src_ap = po[:, j, 0:W]
                        if (c0 // 3) % 2 == 0:
                            nc.scalar.activation(
                                dst, src_ap, AF.Copy,
                                bias=float(delta[c]), scale=rz[:, j : j + 1],
                            )
                        else:
                            nc.vector.tensor_scalar(
                                dst, src_ap, rz[:, j : j + 1], float(delta[c]),
                                OP.mult, OP.add,
                            )

            for sg in range(32):  # 4-channel score groups
                pss = ps_a.tile([H, 4 * H], fp32, tag="ps")
                for j in range(4):
                    c = 4 * sg + j
                    nc.tensor.matmul(
                        pss[:, j * H : (j + 1) * H],
                        qk4[:, 1, c], qk4[:, 0, c],
                        start=(j == 0), stop=(j == 3),
                    )
                et = p_e.tile([H, 4 * H], bf16, tag="e4")
                nc.scalar.activation(et[:], pss[:], AF.Exp, scale=scale_val)
                e_tiles[sg] = et
                # drain o-groups whose channels are fully exp'd (lag 1 group)
                while (next_og < len(o_groups)
                       and o_groups[next_og][0] + o_groups[next_og][1] <= 4 * sg):
                    emit_o_group(*o_groups[next_og])
                    next_og += 1
            while next_og < len(o_groups):
                emit_o_group(*o_groups[next_og])
                next_og += 1

            # ===== Phase G: o -> oT -> conv -> sigmoid/BN/residual =====
            x1f_tiles = {}
            for w0 in range(0, W, 4):
                it = w0 // 4
                if it % 2 == 0:
                    x1f = p_x1f.tile([H, 8 * C], fp32, tag="x1f")
                    nc.sync.dma_start(x1f[:], x1_ap[:, w0 : w0 + 8, :])
                    x1f_tiles[it // 2] = x1f
                x1f_half = x1f_tiles[it // 2][:, (it % 2) * 4 * C : (it % 2 + 1) * 4 * C]

                pst = transpose4(
                    lambda j: o_sb[:, (w0 + j) * C : (w0 + j + 1) * C]
                )
                xt = p_xT.tile([C, 512], bf16, tag="xT")
                evac(xt[:], pst[:], "dve")
                psg = ps_a.tile([H, 512], fp32, tag="ps")
                for j in range(4):
                    nc.tensor.matmul(
                        psg[:, j * C : (j + 1) * C],
                        xt[:, j * C : (j + 1) * C], ws[:],
                        start=(j == 0), stop=(j == 3),
                    )
                if bias_via_dve:
                    nc.vector.tensor_tensor(psg[:], psg[:], bsrep[:], OP.add)
                g4 = p_g.tile([H, 512], bf16, tag="g4")
                nc.scalar.activation(g4[:], psg[:], AF.Sigmoid)
                nc.vector.tensor_tensor(g4[:], g4[:], arep[:], OP.mult)
                if not b_zero:
                    nc.vector.tensor_tensor(g4[:], g4[:], brep[:], OP.add)
                x2slice = x2b[w0 // 8][:, (w0 % 8) * C : (w0 % 8 + 4) * C]
                t4 = p_out.tile([H, 512], fp32, tag="t4")
                if it % 2 == 0:
                    nc.vector.tensor_tensor(t4[:], x2slice, g4[:], OP.mult)
                    nc.gpsimd.tensor_tensor(t4[:], t4[:], x1f_half, OP.add)
                else:
                    nc.gpsimd.tensor_tensor(t4[:], x2slice, g4[:], OP.mult)
                    nc.vector.tensor_tensor(t4[:], t4[:], x1f_half, OP.add)
                nc.sync.dma_start(out_ap[:, w0 : w0 + 4, :], t4[:])

    nc.compile()
    return nc


def _prepare(inputs):
    """Host-side prep: derived small tensors + baked scalars."""
    x1 = np.ascontiguousarray(np.asarray(inputs["x1"], dtype=np.float32))
    x2 = np.ascontiguousarray(np.asarray(inputs["x2"], dtype=np.float32))
    Wq = np.asarray(inputs["Wq"], dtype=np.float32)
    Wk = np.asarray(inputs["Wk"], dtype=np.float32)
    Wv = np.asarray(inputs["Wv"], dtype=np.float32)
    Ws = np.asarray(inputs["Ws"], dtype=np.float32)
    bs = np.asarray(inputs["bs"], dtype=np.float32)
    scale = float(np.asarray(inputs["scale"]).reshape(-1)[0])
    gamma = np.asarray(inputs["gamma"], dtype=np.float32)
    beta = np.asarray(inputs["beta"], dtype=np.float32)
    mu = np.asarray(inputs["mu"], dtype=np.float32)
    var = np.asarray(inputs["var"], dtype=np.float32)

    a = gamma / np.sqrt(var + BN_EPS)
    b = beta - mu * a
    b_zero = bool(np.all(b == 0.0))

    # fold the sigmoid bias bs into o:  o' = o + delta with Ws^T delta = bs
    bias_via_dve = False
    delta = np.zeros(C, dtype=np.float64)
    if np.any(bs != 0.0):
        try:
            delta = np.linalg.solve(Ws.astype(np.float64).T, bs.astype(np.float64))
            resid = np.abs(Ws.T @ delta.astype(np.float32) - bs).max()
            if not np.isfinite(delta).all() or resid > 1e-5 * (1 + np.abs(bs).max()):
                raise np.linalg.LinAlgError("bad solve")
        except np.linalg.LinAlgError:
            delta = np.zeros(C, dtype=np.float64)
            bias_via_dve = True

    bf = ml_dtypes.bfloat16
    consts = {
        "wqk": np.concatenate([Wq, Wk], axis=1).astype(bf),
        "wv": Wv.astype(bf),
        "ws": Ws.astype(bf),
        "ident": np.eye(C, dtype=bf),
        "a_rep": np.tile(a, (C, 4)).astype(bf),
        "b_rep": np.tile(b, (C, 4)).astype(bf),
        "bs_rep": np.tile(bs, (C, 4)).astype(np.float32),
    }
    key = (scale, tuple(np.round(delta, 12)), bias_via_dve, b_zero)
    return x1, x2, consts, key, scale, delta, bias_via_dve, b_zero


def _get_nc(key, scale, delta, bias_via_dve, b_zero):
    if key not in _BUILD_CACHE:
        _BUILD_CACHE[key] = _build_program(scale, delta, bias_via_dve, b_zero)
    return _BUILD_CACHE[key]


def run(inputs, trace: bool = False):
    from concourse.bass_utils import run_bass_kernel_spmd

    x1, x2, consts, key, scale, delta, bias_via_dve, b_zero = _prepare(inputs)
    nc = _get_nc(key, scale, delta, bias_via_dve, b_zero)

    in_maps = []
    for core in range(N_CORES):
        m = dict(consts)
        m["x1"] = x1[core]
        m["x2"] = x2[core]
        in_maps.append(m)

    res = run_bass_kernel_spmd(
        nc, in_maps, core_ids=list(range(N_CORES)), trace=trace
    )
    out = np.stack([res.results[i]["out"] for i in range(N_CORES)], axis=0)
    return out.astype(np.float32), res


def kernel(**inputs) -> np.ndarray:
    out, _ = run(inputs, trace=False)
    return out


# revision 12
# speedup vs baseline: 1.2670x; 1.2651x over previous
"""Trainium2 Bass kernel for nn_CCA_Block (cross-channel attention block).

Reference computation (per batch element, B=8 sharded one-per-core):
    q = relu(x1 @ Wq); k = relu(x1 @ Wk); v = relu(x2 @ Wv)      # 1x1 convs
    scores[c,h,g] = scale * sum_w q[h,w,c] * k[g,w,c]
    attn = softmax(scores, axis=g)
    o[h,w,c] = sum_g attn[c,h,g] * v[g,w,c]
    g = sigmoid(o @ Ws + bs)
    g = gamma * (g - mu) / sqrt(var + eps) + beta
    out = x1 + x2 * g

Sharding: data-parallel over batch across the 8 NeuronCores (batch b -> core b).

Per-core dataflow (v3):
  All HBM traffic on HWDGE (fp32 staging + wide contiguous engine casts;
  SWDGE cast-DMAs cost ~1.3us of Q7 time each and gpsimd tensor ops run
  at ~2.5ns/elem, so gpsimd stays off the hot path entirely).
  All PSUM evacuations use CONTIGUOUS free-dim APs (strided evacs measure
  ~5ns/elem vs ~0.7ns contiguous).
  QK+V interleaved per 8-pixel chunk:
    x1 staged w-major -> bf16 cast xb; x2 staged h-major -> bf16 cast
    into resident chunk x2b[i] (reused by V transposes AND the phase-G
    residual; x2 is read from HBM exactly once).
    PE tile transposes (bf16, 4 per psum bank) -> convs with the weight
    as the moving operand -> relu evacs into qk_sb [w, (h,s,c)] /
    v_sb [g, w*C+c | ones-block].
  A:  4-channel score groups per PSUM bank -> ONE wide exp per bank
      (amortizes the 352-cycle ACT fixed cost 4x) -> e tiles [g, (c,h)]
      (contiguous o-matmul lhsT) -> per-channel o matmul, rhs = strided
      v column-slice with trailing ones column (N=129, softmax
      denominator for free) -> 1/Z (DVE) -> wide contiguous normalize
      into o_sb [h, (c,w)].
  G:  oT transpose -> conv with Ws -> sigmoid -> BN affine ->
      t = x2b*g + x1c (x1 re-read bf16: staged fp32 + ACT cast,
      prefetchable during A) -> out store.
"""

import numpy as np
import ml_dtypes

B, H, W, C = 8, 128, 128, 128
N_CORES = 8
BN_EPS = 1e-3

_BUILD_CACHE: dict = {}


def _build_program(scale_val: float, delta: tuple, bias_via_dve: bool, b_zero: bool):
    """Emit + compile the per-core Bass program. All cores run the identical
    program on their own batch slice."""
    import concourse.bacc as bacc
    import concourse.mybir as mybir
    import concourse.tile as tile

    fp32 = mybir.dt.float32
    bf16 = mybir.dt.bfloat16
    AF = mybir.ActivationFunctionType
    OP = mybir.AluOpType
    delta_zero = all(d == 0.0 for d in delta)

    nc = bacc.Bacc("TRN2", target_bir_lowering=False, debug=False,
                   enable_asserts=False)

    x1_d = nc.dram_tensor("x1", [H, W, C], fp32, kind="ExternalInput")
    x2_d = nc.dram_tensor("x2", [H, W, C], fp32, kind="ExternalInput")
    wqk_d = nc.dram_tensor("wqk", [C, 2 * C], bf16, kind="ExternalInput")
    wv_d = nc.dram_tensor("wv", [C, C], bf16, kind="ExternalInput")
    ws_d = nc.dram_tensor("ws", [C, C], bf16, kind="ExternalInput")
    ident_d = nc.dram_tensor("ident", [C, C], bf16, kind="ExternalInput")
    arep_d = nc.dram_tensor("a_rep", [C, 4 * C], bf16, kind="ExternalInput")
    brep_d = nc.dram_tensor("b_rep", [C, 4 * C], bf16, kind="ExternalInput")
    bsrep_d = nc.dram_tensor("bs_rep", [C, 4 * C], fp32, kind="ExternalInput")
    out_d = nc.dram_tensor("out", [H, W, C], fp32, kind="ExternalOutput")

    x1_ap, x2_ap, out_ap = x1_d.ap(), x2_d.ap(), out_d.ap()

    with tile.TileContext(nc) as tc:
        with (
            # persistent single-buffer pools
            tc.tile_pool(name="wts", bufs=1) as p_wts,
            tc.tile_pool(name="qkv", bufs=1) as p_qkv,
            tc.tile_pool(name="obuf", bufs=1) as p_o,
            tc.tile_pool(name="x2res", bufs=16) as p_x2b,
            # streaming pools
            tc.tile_pool(name="stage", bufs=3) as p_stage,     # fp32 staging
            tc.tile_pool(name="xcast", bufs=3) as p_xcast,     # x1 bf16 chunks
            tc.tile_pool(name="xT", bufs=4) as p_xT,
            tc.tile_pool(name="eexp", bufs=4) as p_e,
            tc.tile_pool(name="rz", bufs=6) as p_rz,
            tc.tile_pool(name="gres", bufs=3) as p_g,
            tc.tile_pool(name="x1c", bufs=6) as p_x1c,         # bf16 residual
            tc.tile_pool(name="outt", bufs=2) as p_out,
            # psum
            tc.tile_pool(name="psA", bufs=5, space="PSUM") as ps_a,
            tc.tile_pool(name="psT", bufs=3, space="PSUM") as ps_t,
        ):
            # ---- constants ----
            wqk = p_wts.tile([C, 2 * C], bf16, tag="wqk")
            wv = p_wts.tile([C, C], bf16, tag="wv")
            ws = p_wts.tile([C, C], bf16, tag="ws")
            ident = p_wts.tile([C, C], bf16, tag="ident")
            arep = p_wts.tile([C, 4 * C], bf16, tag="arep")
            nc.sync.dma_start(wqk[:], wqk_d.ap())
            nc.sync.dma_start(wv[:], wv_d.ap())
            nc.sync.dma_start(ws[:], ws_d.ap())
            nc.sync.dma_start(ident[:], ident_d.ap())
            nc.sync.dma_start(arep[:], arep_d.ap())
            if not b_zero:
                brep = p_wts.tile([C, 4 * C], bf16, tag="brep")
                nc.sync.dma_start(brep[:], brep_d.ap())
            if bias_via_dve:
                bsrep = p_wts.tile([C, 4 * C], fp32, tag="bsrep")
                nc.sync.dma_start(bsrep[:], bsrep_d.ap())

            # persistent big buffers (free-dim layouts noted)
            qk_sb = p_qkv.tile([W, 2 * C * H], bf16, tag="qk")  # [w, h*256+s*128+c]
            # v plus a trailing ones-block: column W*C+c == 1.0 so a single
            # N=129 strided matmul computes o_unnorm and the softmax denom Z
            v_sb = p_qkv.tile([H, W * C + C], bf16, tag="v")    # [g, w*128+c]
            nc.vector.memset(v_sb[:, W * C :], 1.0)
            o_sb = p_o.tile([H, C * W], bf16, tag="o")          # [h, c*128+w]
            # x2 bf16 resident chunks: chunk i holds x2[:, 8i:8i+8, :]
            x2b = [
                p_x2b.tile([H, 8 * C], bf16, tag="x2b", name=f"x2b{i}")
                for i in range(16)
            ]

            qk4 = qk_sb[:].rearrange("w (h s c) -> w h s c", s=2, c=C)
            o3 = o_sb[:].rearrange("h (c w) -> h c w", w=W)

            def transpose4(src_fn):
                """4 PE tile-transposes into one bf16 PSUM bank.
                src_fn(j) -> [128,128] bf16 SBUF AP. Returns PSUM tile."""
                pst = ps_t.tile([C, 512], bf16, tag="pst")
                for j in range(4):
                    nc.tensor.matmul(
                        pst[:, j * C : (j + 1) * C], src_fn(j), ident[:],
                        is_transpose=True, start=(j == 0), stop=(j == 3),
                    )
                return pst

            def evac(dst, src, engine):
                if engine == "act":
                    nc.scalar.activation(dst, src, AF.Copy)
                else:
                    nc.vector.tensor_copy(dst, src)

            def relu_evac(dst, src, engine):
                if engine == "act":
                    nc.scalar.activation(dst, src, AF.Relu)
                else:
                    nc.vector.tensor_scalar(dst, src, 0.0, None, OP.max)

            # ===== Phases QK and V, interleaved per 8-pixel chunk =====
            for i in range(16):
                h0 = 8 * i
                # x1 staged w-major for the QK path
                stg = p_stage.tile([W, 8 * C], fp32, tag="stg")
                nc.sync.dma_start(
                    stg[:], x1_ap[h0 : h0 + 8].rearrange("hh w c -> w hh c")
                )
                xb = p_xcast.tile([W, 8 * C], bf16, tag="xb")
                evac(xb[:], stg[:], "act" if i % 2 == 0 else "dve")
                # x2 staged h-major; bf16 resident chunk feeds V transposes
                # now and the phase-G residual later (single HBM read)
                stg2 = p_stage.tile([H, 8 * C], fp32, tag="stg")
                nc.sync.dma_start(stg2[:], x2_ap[:, h0 : h0 + 8, :])
                evac(x2b[i][:], stg2[:], "dve" if i % 2 == 0 else "act")

                for j2 in range(2):  # QK: two 4-h subgroups
                    hh = 4 * j2
                    pst = transpose4(lambda j: xb[:, (hh + j) * C : (hh + j + 1) * C])
                    xt = p_xT.tile([C, 512], bf16, tag="xT")
                    evac(xt[:], pst[:], "act" if j2 == 0 else "dve")
                    for s2 in range(2):
                        psqk = ps_a.tile([W, 512], fp32, tag="ps")
                        for t in range(2):
                            nc.tensor.matmul(
                                psqk[:, t * 256 : (t + 1) * 256],
                                xt[:, (2 * s2 + t) * C : (2 * s2 + t + 1) * C],
                                wqk[:], start=(t == 0), stop=(t == 1),
                            )
                        h = h0 + hh + 2 * s2
                        dst = qk_sb[:, h * 2 * C : (h + 2) * 2 * C]
                        relu_evac(dst, psqk[:], ("dve", "act")[(2 * j2 + s2) % 2])

                for j2 in range(2):  # V: two 4-w subgroups
                    ww = 4 * j2
                    pst = transpose4(
                        lambda j: x2b[i][:, (ww + j) * C : (ww + j + 1) * C]
                    )
                    xt = p_xT.tile([C, 512], bf16, tag="xT")
                    evac(xt[:], pst[:], "dve" if j2 == 0 else "act")
                    psv = ps_a.tile([H, 512], fp32, tag="ps")
                    for j in range(4):
                        nc.tensor.matmul(
                            psv[:, j * C : (j + 1) * C],
                            xt[:, j * C : (j + 1) * C], wv[:],
                            start=(j == 0), stop=(j == 3),
                        )
                    w0 = h0 + ww
                    relu_evac(
                        v_sb[:, w0 * C : (w0 + 4) * C], psv[:],
                        ("act", "dve")[j2],
                    )

            # ===== Phase A: attention over channels =====
            e_tiles = {}  # sg -> e tile [g, 4H] bf16, channels 4sg..4sg+3
            o_groups = [(c0, min(3, C - c0)) for c0 in range(0, C, 3)]
            next_og = 0

            def emit_o_group(c0, gs):
                pso = ps_a.tile([H, gs * 129], fp32, tag="ps")
                for j in range(gs):
                    c = c0 + j
                    et = e_tiles[c // 4]
                    nc.tensor.matmul(
                        pso[:, j * 129 : (j + 1) * 129],
                        et[:, (c % 4) * H : (c % 4 + 1) * H],
                        v_sb[:, c : c + W * C + 1 : C],
                        start=(j == 0), stop=(j == gs - 1),
                    )
                po = pso[:].rearrange("h (j x) -> h j x", x=129)
                rz = p_rz.tile([H, gs], fp32, tag="rz")
                nc.vector.reciprocal(rz[:], po[:, :, 128])
                if delta_zero:
                    # wide normalize: o = o_unnorm * (1/Z) with 1/Z
                    # broadcast along w; contiguous-inner dst and src
                    rzb = rz[:].unsqueeze(2).broadcast_to([H, gs, W])
                    nc.vector.tensor_tensor(
                        o3[:, c0 : c0 + gs, :], po[:, :, 0:W], rzb, OP.mult,
                    )
                else:
                    for j in range(gs):
                        c = c0 + j
                        dst = o3[:, c, :]
                        src_ap = po[:, j, 0:W]
                        if (c0 // 3) % 2 == 0:
                            nc.scalar.activation(
                                dst, src_ap, AF.Copy,
                                bias=float(delta[c]), scale=rz[:, j : j + 1],
                            )
                        else:
                            nc.vector.tensor_scalar(
                                dst, src_ap, rz[:, j : j + 1], float(delta[c]),
                                OP.mult, OP.add,
                            )

            for sg in range(32):  # 4-channel score groups
                pss = ps_a.tile([H, 4 * H], fp32, tag="ps")
                for j in range(4):
                    c = 4 * sg + j
                    nc.tensor.matmul(
                        pss[:, j * H : (j + 1) * H],
                        qk4[:, :, 1, c], qk4[:, :, 0, c],
                        start=(j == 0), stop=(j == 3),
                    )
                et = p_e.tile([H, 4 * H], bf16, tag="e4")
                nc.scalar.activation(et[:], pss[:], AF.Exp, scale=scale_val)
                e_tiles[sg] = et
                # drain o-groups whose channels are all exp'd already
                while (next_og < len(o_groups)
                       and o_groups[next_og][0] + o_groups[next_og][1] <= 4 * sg):
                    emit_o_group(*o_groups[next_og])
                    next_og += 1
            while next_og < len(o_groups):
                emit_o_group(*o_groups[next_og])
                next_og += 1

            # ===== Phase G: o -> oT -> conv -> sigmoid/BN/residual =====
            x1c_tiles = {}
            for w0 in range(0, W, 4):
                it = w0 // 4
                if it % 2 == 0:
                    stg3 = p_stage.tile([H, 8 * C], fp32, tag="stg")
                    nc.sync.dma_start(stg3[:], x1_ap[:, w0 : w0 + 8, :])
                    x1c = p_x1c.tile([H, 8 * C], bf16, tag="x1c")
                    evac(x1c[:], stg3[:], "act")
                    x1c_tiles[it // 2] = x1c
                x1ch = x1c_tiles[it // 2][:, (it % 2) * 4 * C : (it % 2 + 1) * 4 * C]

                pst = transpose4(lambda j: o3[:, :, w0 + j])
                xt = p_xT.tile([C, 512], bf16, tag="xT")
                evac(xt[:], pst[:], "dve")
                psg = ps_a.tile([H, 512], fp32, tag="ps")
                for j in range(4):
                    nc.tensor.matmul(
                        psg[:, j * C : (j + 1) * C],
                        xt[:, j * H : (j + 1) * H], ws[:],
                        start=(j == 0), stop=(j == 3),
                    )
                if bias_via_dve:
                    nc.vector.tensor_tensor(psg[:], psg[:], bsrep[:], OP.add)
                g4 = p_g.tile([H, 512], bf16, tag="g4")
                nc.scalar.activation(g4[:], psg[:], AF.Sigmoid)
                nc.vector.tensor_tensor(g4[:], g4[:], arep[:], OP.mult)
                if not b_zero:
                    nc.vector.tensor_tensor(g4[:], g4[:], brep[:], OP.add)
                x2slice = x2b[w0 // 8][:, (w0 % 8) * C : (w0 % 8 + 4) * C]
                t4 = p_out.tile([H, 512], fp32, tag="t4")
                nc.vector.tensor_tensor(t4[:], x2slice, g4[:], OP.mult)
                nc.vector.tensor_tensor(t4[:], t4[:], x1ch, OP.add)
                nc.sync.dma_start(out_ap[:, w0 : w0 + 4, :], t4[:])

    nc.compile()
    return nc


def _prepare(inputs):
    """Host-side prep: derived small tensors + baked scalars."""
    x1 = np.ascontiguousarray(np.asarray(inputs["x1"], dtype=np.float32))
    x2 = np.ascontiguousarray(np.asarray(inputs["x2"], dtype=np.float32))
    Wq = np.asarray(inputs["Wq"], dtype=np.float32)
    Wk = np.asarray(inputs["Wk"], dtype=np.float32)
    Wv = np.asarray(inputs["Wv"], dtype=np.float32)
    Ws = np.asarray(inputs["Ws"], dtype=np.float32)
    bs = np.asarray(inputs["bs"], dtype=np.float32)
    scale = float(np.asarray(inputs["scale"]).reshape(-1)[0])
    gamma = np.asarray(inputs["gamma"], dtype=np.float32)
    beta = np.asarray(inputs["beta"], dtype=np.float32)
    mu = np.asarray(inputs["mu"], dtype=np.float32)
    var = np.asarray(inputs["var"], dtype=np.float32)

    a = gamma / np.sqrt(var + BN_EPS)
    b = beta - mu * a
    b_zero = bool(np.all(b == 0.0))

    # fold the sigmoid bias bs into o:  o' = o + delta with Ws^T delta = bs
    bias_via_dve = False
    delta = np.zeros(C, dtype=np.float64)
    if np.any(bs != 0.0):
        try:
            delta = np.linalg.solve(Ws.astype(np.float64).T, bs.astype(np.float64))
            resid = np.abs(Ws.T @ delta.astype(np.float32) - bs).max()
            if not np.isfinite(delta).all() or resid > 1e-5 * (1 + np.abs(bs).max()):
                raise np.linalg.LinAlgError("bad solve")
        except np.linalg.LinAlgError:
            delta = np.zeros(C, dtype=np.float64)
            bias_via_dve = True

    bf = ml_dtypes.bfloat16
    consts = {
        "wqk": np.concatenate([Wq, Wk], axis=1).astype(bf),
        "wv": Wv.astype(bf),
        "ws": Ws.astype(bf),
        "ident": np.eye(C, dtype=bf),
        "a_rep": np.tile(a, (C, 4)).astype(bf),
        "b_rep": np.tile(b, (C, 4)).astype(bf),
        "bs_rep": np.tile(bs, (C, 4)).astype(np.float32),
    }
    key = (scale, tuple(np.round(delta, 12)), bias_via_dve, b_zero)
    return x1, x2, consts, key, scale, delta, bias_via_dve, b_zero


def _get_nc(key, scale, delta, bias_via_dve, b_zero):
    if key not in _BUILD_CACHE:
        _BUILD_CACHE[key] = _build_program(scale, delta, bias_via_dve, b_zero)
    return _BUILD_CACHE[key]


def run(inputs, trace: bool = False):
    from concourse.bass_utils import run_bass_kernel_spmd

    x1, x2, consts, key, scale, delta, bias_via_dve, b_zero = _prepare(inputs)
    nc = _get_nc(key, scale, delta, bias_via_dve, b_zero)

    in_maps = []
    for core in range(N_CORES):
        m = dict(consts)
        m["x1"] = x1[core]
        m["x2"] = x2[core]
        in_maps.append(m)

    res = run_bass_kernel_spmd(
        nc, in_maps, core_ids=list(range(N_CORES)), trace=trace
    )
    out = np.stack([res.results[i]["out"] for i in range(N_CORES)], axis=0)
    return out.astype(np.float32), res


def kernel(**inputs) -> np.ndarray:
    out, _ = run(inputs, trace=False)
    return out
